# revision 3
# baseline (speedup 1.0000x reference)
"""Gumbel-softmax palette quantization on 8 TRN2 NeuronCores.

Math (per batch b, pixel p, palette entry k):
    gumbel = -ln(-ln(u + eps) + eps)
    probs  = softmax((img + gumbel) / T, axis=k)
    out    = probs @ palette                      # [pix, 4]

Rewrite used here (softmax is scale invariant per pixel; no max-subtract
needed since z = (img+gumbel)/T <= ~25 and exp stays in fp32 range):
    G   = ln(eps - ln(u + eps))        # = -gumbel     (2 ACT passes, one table set)
    z   = img - G                      # DVE subtract
    e   = exp(z / T)                   # ACT, PSUM->SBUF, cast to bf16
    out[p, c] = (e @ pal_aug)[p, c] / (e @ pal_aug)[p, 4]
where pal_aug = [palette | ones] so the TensorE contraction produces the
softmax denominator for free.

Sharding: data-parallel over batch, 1 batch per core (b=8, 8 cores).

Layout per core: 64 chunks of [128 part, 2048 free] fp32; partition p of
chunk c holds pixels c*1024 + p*8 .. +8 (8 pixels x 256 k contiguous), so
all HBM traffic is fully contiguous. The k-contraction needs k on
partitions, so each [128 pix, 128 k-half] block of z is transposed on the
TensorEngine (transpose-mode matmul with identity, fp32 = 2 cyc/row) into
PSUM; the ACT exp pass reads PSUM and writes bf16 e^T to SBUF (doubling as
the PSUM->SBUF move); the palette matmul (lhsT = e^T block, rhs = pal_aug
bf16 [k,5]) accumulates the two k-halves into a [128 pix, 5] PSUM tile.
"""

import numpy as np
import ml_dtypes

B, H, W, K, C = 8, 256, 256, 256, 4
NPIX = H * W                       # 65536 pixels per batch/core
FD = 2048                          # free dim per chunk
PPP = FD // K                      # 8 pixels per partition per chunk
NCHUNK = NPIX * K // (128 * FD)    # 64 chunks
EPS = 1e-20
NCORES = 8

_cache: dict = {}


def _build(temp: float):
    import concourse.mybir as mybir
    from concourse import bacc
    from concourse.tile import TileContext

    dt = mybir.dt
    AF = mybir.ActivationFunctionType

    nc = bacc.Bacc(
        "TRN2", target_bir_lowering=False, debug=False, num_devices=NCORES
    )

    img_d = nc.dram_tensor("images", [NCHUNK, 128, FD], dt.float32, kind="ExternalInput")
    noi_d = nc.dram_tensor("noise", [NCHUNK, 128, FD], dt.float32, kind="ExternalInput")
    pal_d = nc.dram_tensor("pal", [128, 2, 5], dt.bfloat16, kind="ExternalInput")
    idn_d = nc.dram_tensor("ident", [128, 128], dt.float32, kind="ExternalInput")
    out_d = nc.dram_tensor("out", [NCHUNK, 128, PPP * 4], dt.float32, kind="ExternalOutput")

    with TileContext(nc) as tc:
        with (
            tc.tile_pool(name="const", bufs=1) as cpool,
            tc.tile_pool(name="img", bufs=3) as ipool,
            tc.tile_pool(name="noi", bufs=3) as npool,
            tc.tile_pool(name="et", bufs=3) as epool,
            tc.tile_pool(name="epi", bufs=2) as xpool,
            tc.tile_pool(name="outp", bufs=3) as opool,
            tc.tile_pool(name="zt", bufs=2, space="PSUM") as ztpool,
            tc.tile_pool(name="acc", bufs=2, space="PSUM") as accpool,
        ):
            ident = cpool.tile([128, 128], dt.float32, tag="ident")
            nc.sync.dma_start(ident[:], idn_d[:])
            pal = cpool.tile([128, 2, 5], dt.bfloat16, tag="pal")
            nc.sync.dma_start(pal[:], pal_d[:])
            epsb = cpool.tile([128, 1], dt.float32, tag="epsb")
            nc.vector.memset(epsb[:], EPS)

            for ci in range(NCHUNK):
                img = ipool.tile([128, FD], dt.float32)
                noi = npool.tile([128, FD], dt.float32)
                nc.sync.dma_start(img[:], img_d[ci])
                nc.sync.dma_start(noi[:], noi_d[ci])
                # L = ln(u + eps); G = ln(eps - L)   (in-place, same table set)
                nc.scalar.activation(noi[:], noi[:], AF.Ln, bias=epsb[:])
                nc.scalar.activation(noi[:], noi[:], AF.Ln, bias=epsb[:], scale=-1.0)
                # z = img - G  (in-place on img)
                nc.vector.tensor_sub(img[:], img[:], noi[:])

                acc = accpool.tile([128, PPP * 5], dt.float32)
                for g in range(2):  # 2 groups of 4 pixel-columns
                    zt = ztpool.tile([128, 1024], dt.float32)
                    for jl in range(4):
                        jj = g * 4 + jl
                        for h in range(2):
                            nc.tensor.transpose(
                                zt[:, (jl * 2 + h) * 128:(jl * 2 + h + 1) * 128],
                                img[:, jj * 256 + h * 128: jj * 256 + (h + 1) * 128],
                                ident[:],
                            )
                    et = epool.tile([128, 1024], dt.bfloat16)
                    nc.scalar.activation(et[:], zt[:], AF.Exp, scale=1.0 / temp)
                    for jl in range(4):
                        jj = g * 4 + jl
                        for h in range(2):
                            nc.tensor.matmul(
                                acc[:, jj * 5:(jj + 1) * 5],
                                et[:, (jl * 2 + h) * 128:(jl * 2 + h + 1) * 128],
                                pal[:, h, :],
                                start=(h == 0),
                                stop=(h == 1),
                            )

                # epilogue: divide colors by the ones-column sum
                raw = xpool.tile([128, PPP * 5], dt.float32, tag="raw")
                nc.vector.tensor_copy(raw[:], acc[:])
                rv = raw[:].rearrange("p (j c) -> p j c", c=5)
                sinv = xpool.tile([128, PPP], dt.float32, tag="sinv")
                nc.vector.tensor_copy(sinv[:], rv[:, :, 4])
                nc.vector.reciprocal(sinv[:], sinv[:])
                outf = opool.tile([128, PPP * 4], dt.float32)
                ov = outf[:].rearrange("p (j c) -> p j c", c=4)
                for cc in range(4):
                    nc.vector.tensor_mul(ov[:, :, cc], rv[:, :, cc], sinv[:])
                nc.sync.dma_start(out_d[ci], outf[:])

    nc.compile()
    return nc


def _get_nc(temp: float):
    if temp not in _cache:
        _cache[temp] = _build(temp)
    return _cache[temp]


def _make_in_maps(images, palettes, uniform_noise):
    ident = np.eye(128, dtype=np.float32)
    in_maps = []
    for i in range(NCORES):
        aug = np.concatenate(
            [palettes[i].astype(np.float32), np.ones((K, 1), np.float32)], axis=1
        )  # [256, 5]
        pal = np.ascontiguousarray(
            aug.reshape(2, 128, 5).transpose(1, 0, 2)
        ).astype(ml_dtypes.bfloat16)  # [128(k_lo), 2(k_hi), 5]
        in_maps.append(
            {
                "images": np.ascontiguousarray(images[i]).reshape(NCHUNK, 128, FD),
                "noise": np.ascontiguousarray(uniform_noise[i]).reshape(NCHUNK, 128, FD),
                "pal": pal,
                "ident": ident,
            }
        )
    return in_maps


def _unshard(results):
    outs = []
    for i in range(NCORES):
        o = np.asarray(results[i]["out"], dtype=np.float32)  # [NCHUNK,128,PPP*4]
        outs.append(o.reshape(NPIX, 4).reshape(H, W, 4))
    return np.stack(outs)  # [8, 256, 256, 4]


def kernel(**inputs) -> np.ndarray:
    from concourse.bass_utils import run_bass_kernel_spmd

    images = np.asarray(inputs["images"], dtype=np.float32)
    palettes = np.asarray(inputs["palettes"], dtype=np.float32)
    noise = np.asarray(inputs["uniform_noise"], dtype=np.float32)
    temp = float(np.asarray(inputs["temperature"]))

    nc = _get_nc(temp)
    in_maps = _make_in_maps(images, palettes, noise)
    res = run_bass_kernel_spmd(nc, in_maps, list(range(NCORES)))
    return _unshard(res.results)
